# revision 29
# baseline (speedup 1.0000x reference)
"""Trainium2 Bass kernel for the all-pairs DFT-D3 dispersion energy sum.

Math: energy = -sum_{i!=j} f(d2_ij),  f(x) = s6/(x^3+c6) + s8/(x^4+c8),
d2 = |p_i - p_j|^2.  atomic_numbers / r2r4 are multiplied by 0.0 in the
reference -> ignored.

Approximations (error budget 2e-2; total measured ~2.7e-3):
  * f(x) ~= 1/h(x), h cubic (fit weighted by pair density*contribution).
  * Atoms z-sorted into 16 slabs of 512; block pairs >2 slabs apart
    dropped.  45 real blocks + 3 dummies = 48 -> 6 per core.
  * 1/H on VE in one op: y0 = bitcast(~H), m = H*y0 in [-4.5,-4],
    1/H = y0*cheb2(m), fused accum over the free dim.
  * Delta=2 blocks (x>=14) routed entirely through ScalarE as
    w*alpha*((x+beta)^2+gamma)^(-p) = Square -> Ln -> Exp(accum).

Per-core schedule (KPC=6): k0 = Delta2 block, SE-only (Square/Ln/Exp
straight from PSUM - zero VE work); k1,k2 = VE poly -> fused SE Ln/Exp
(one Ln + one Exp over both blocks); k3 = off-diag VE poly+recip;
k4,k5 = diag VE (w=1).  VE runs 8 passes back-to-back; PE (bf16 K=13
hi/lo-split features, exact products) stays ~2 blocks ahead; ScalarE
fills the VE window.  Host: z-sort/gather, fp64 strip sum, subtract N
unmasked diagonal terms, negate.
"""

import numpy as np

N = 8192
BLK = 512
NBLK = N // BLK          # 16 z-slabs
NCORES = 8
DMAX = 2                 # keep block pairs |bi-bj| <= DMAX
KPC = 6
FD = 2048                # 4 j-subtiles x 512 i-cols per PSUM tile
NFEAT = 13

# cubic fit of 1/f:  1/f ~= a3*x^3 + a2*x^2 + a1*x + a0
CUB_A3 = 1.0000149181184413
CUB_A2 = -0.7417928143850965
CUB_A1 = -0.9251683748940465
CUB_A0 = 12929.698617787097
# deg-2 Chebyshev of 1/m on m in [-4.5, -4.0]
RC0 = -0.7071068235208974
RC1 = -0.1665221267860314
RC2 = -0.0130605626142685
# SE-only model for x>=14:  f ~= exp(SE_LNA) * ((x+SE_BETA)^2+SE_GAMMA)^-SE_P
SE_BETA = -6.64555739708305
SE_GAMMA = 486.26339501988436
SE_LNA = -0.33730306582730434
SE_P = 1.4769639633186915

_OPS = {}


def _register_ops():
    """POLY3_HORNER_ANT:    out = ((x+C0)*x+C1)*x+C2
       RECIP_CHEB2_ACC_ANT: out = y0*(C0 + m*(C1 + m*C2)), y0=bitcast(~x),
                            m = x*y0; accum_out = sum(out) over free dim."""
    if _OPS:
        return _OPS
    import operator

    from concourse import dve_ops
    from concourse.dve_spec import C0, C1, C2, Bin, Spec, Src0, lower
    from concourse.dve_uop import AluOp, DveOpSpec

    def bitnot_np(x):
        return (~x.view(np.int32)).view(np.float32)

    def mk(name, spec):
        if name in dve_ops._SUB_OPCODE_FOR_NAME:
            return next(o for o in dve_ops.OPS if o.name == name)
        row = dve_ops._CUSTOM_DVE_ROW_BASE + len(dve_ops.OPS)
        assert row < 0x20
        dve_ops._SUB_OPCODE_FOR_NAME[name] = row
        shas = {}
        for ver in ("v3", "v4"):
            uops = lower(spec, ver=ver)
            shas[ver] = DveOpSpec(
                name=name, opcode=row, uops=uops, rd1_en=False
            ).sha(ver)
        op = dve_ops.DveOp(name, spec, subdim=False, uops_sha=shas)
        dve_ops.OPS.append(op)
        dve_ops.CUSTOM_DVE_SPECS[name] = spec
        return op

    def poly3_ref(in0, in1, c0, c1, c2):
        x = in0.astype(np.float32)
        return (((x + np.float32(c0)) * x + np.float32(c1)) * x
                + np.float32(c2)).astype(np.float32)

    _OPS["poly3"] = mk(
        "POLY3_HORNER_ANT",
        Spec(body=((Src0 + C0) * Src0 + C1) * Src0 + C2, reference=poly3_ref),
    )

    y0 = Bin(AluOp.BITWISE_NOT, Src0, Src0)
    m = Src0 * y0
    body = y0 * (C0 + m * (C1 + m * C2))

    def recip_ref(in0, in1, c0, c1, c2):
        x = in0.astype(np.float32)
        yy = bitnot_np(x)
        mm = (x * yy).astype(np.float32)
        out = (yy * (np.float32(c0) + mm * (np.float32(c1) + mm * np.float32(c2)))
               ).astype(np.float32)
        return out, out.sum(axis=-1, keepdims=True, dtype=np.float32)

    _OPS["recipacc"] = mk(
        "RECIP_CHEB2_ACC_ANT",
        Spec(body=body, accum=operator.add, reference=recip_ref),
    )
    return _OPS


def _recip_cheb_host(h, scale):
    """Bit-exact host replica of RECIP_CHEB2_ACC_ANT with constants scaled
    by `scale` (as baked into the diag instruction)."""
    h = np.asarray(h, np.float32)
    y0 = (~h.view(np.int32)).view(np.float32)
    m = (h * y0).astype(np.float32)
    c0 = np.float32(scale * RC0)
    c1 = np.float32(scale * RC1)
    c2 = np.float32(scale * RC2)
    return (y0 * (c0 + m * (c1 + m * c2))).astype(np.float32)


def _block_lists():
    """6 block pairs per core: k0 off-diag (VE), k1 Delta2 (SE-only:
    Square/Ln/Exp), k2,k3 off-diag (VE poly + SE ln/exp), k4,k5 diag (VE).
    None = dummy."""
    diag = [(b, b) for b in range(NBLK)]
    d1 = [(i, i + 1) for i in range(NBLK - 1)]          # 15
    d2 = [(i, i + 2) for i in range(NBLK - 2)]          # 14
    rest = d2[NCORES:] + d1 + [None] * (3 * NCORES - len(d2[NCORES:]) - len(d1))
    per_core = []
    for c in range(NCORES):
        mine = rest[c::NCORES]
        blocks = [mine[0], d2[c], mine[1], mine[2],
                  diag[2 * c], diag[2 * c + 1]]
        assert len(blocks) == KPC
        per_core.append(blocks)
    return per_core


def _build_program(pA, pB, pC):
    import concourse.mybir as mybir
    from concourse import bacc
    from concourse.tile import TileContext

    ops = _register_ops()
    f32 = mybir.dt.float32
    bf = mybir.dt.bfloat16
    AF = mybir.ActivationFunctionType
    inv_a3 = 1.0 / CUB_A3
    half = 3 * BLK

    nc = bacc.Bacc(None, target_bir_lowering=False, debug=False)
    # column layout: [A(blk 0..2) | B(blk 0..2) | A(blk 3..5) | B(blk 3..5)]
    ab_in = nc.dram_tensor("absel", (NFEAT, 4 * half), bf,
                           kind="ExternalInput")
    out = nc.dram_tensor("out", (128, KPC + 3), f32, kind="ExternalOutput")

    with TileContext(nc) as tc:
        with (
            tc.tile_pool(name="const", bufs=1) as constp,
            tc.tile_pool(name="psum", bufs=2, space="PSUM") as psump,
            tc.tile_pool(name="ab", bufs=1) as abp,
            tc.tile_pool(name="work", bufs=3) as work,
        ):
            stripV = constp.tile([128, KPC], f32, tag="stripV")
            stripS = constp.tile([128, 3], f32, tag="stripS")
            dumpV = constp.tile([128, FD], f32, tag="dumpV")
            dumpS = constp.tile([128, FD], f32, tag="dumpS")
            cb = constp.tile([128, 4], f32, tag="cb")
            u0 = constp.tile([128, FD], f32, tag="u0")
            # one shared tile for all three SE ln outputs: tile-granularity
            # deps force the SE order Sq, Ln, Ln, Ln, Exp, Exp, Exp (2
            # act-table switches instead of one per activation)
            lS = constp.tile([128, 3 * FD], f32, tag="lS")
            nc.vector.memset(stripV[:, :], 0.0)
            nc.vector.memset(stripS[:, :], 0.0)
            nc.gpsimd.memset(cb[:, 0:1], float(np.log(2.0 * inv_a3)))
            nc.gpsimd.memset(cb[:, 1:2], float(SE_LNA + np.log(2.0)))
            nc.gpsimd.memset(cb[:, 2:3], float(SE_BETA))
            nc.gpsimd.memset(cb[:, 3:4], float(SE_GAMMA))
            # preload the Square act table during the preamble so Sq(k1)
            # issues the moment its PSUM is ready
            nc.scalar.activation(dumpS[:, 0:1], cb[:, 0:1], AF.Square,
                                 bias=cb[:, 2:3], scale=1.0)

            AB0 = abp.tile([NFEAT, 2 * half], bf, tag="ab0")
            AB1 = abp.tile([NFEAT, 2 * half], bf, tag="ab1")
            nc.sync.dma_start(AB0[:, :], ab_in[:, :2 * half])
            nc.sync.dma_start(AB1[:, :], ab_in[:, 2 * half:])

            def cols(k):
                t = AB0 if k < 3 else AB1
                a = (k % 3) * BLK
                return t, a, half + a

            for k in range(KPC):
                w = (1.0 if k >= 4 else 2.0) * inv_a3
                T, ao, bo = cols(k)
                psum = psump.tile([128, FD], f32, tag="d2")
                for t in range(4):
                    nc.tensor.matmul(
                        psum[:, t * 512:(t + 1) * 512],
                        T[:, bo + t * 128: bo + (t + 1) * 128],
                        T[:, ao:ao + BLK],
                        start=True, stop=True,
                    )
                if k == 1:
                    # SE-only route: u=(x+beta)^2 straight from PSUM
                    nc.scalar.activation(u0[:, :], psum[:, :], AF.Square,
                                         bias=cb[:, 2:3], scale=1.0)
                    nc.scalar.activation(lS[:, 0:FD], u0[:, :], AF.Ln,
                                         bias=cb[:, 3:4], scale=1.0)
                    continue
                if k in (2, 3):
                    ht = work.tile([128, FD], f32, tag="h")
                    h = ht[:, :]
                    nc.vector._custom_dve(
                        ops["poly3"], out=h, in0=psum[:, :],
                        s0=float(pA), s1=float(pB), imm2=float(pC),
                    )
                    nc.scalar.activation(lS[:, (k - 1) * FD:k * FD], h,
                                         AF.Ln)
                    if k == 3:
                        # all three exps after the last ln (lS tile dep)
                        nc.scalar.activation(
                            dumpS[:, :], lS[:, 0:FD], AF.Exp,
                            bias=cb[:, 1:2], scale=float(-SE_P),
                            accum_out=stripS[:, 0:1],
                        )
                        for kk in (2, 3):
                            nc.scalar.activation(
                                dumpS[:, :], lS[:, (kk - 1) * FD:kk * FD],
                                AF.Exp, bias=cb[:, 0:1], scale=-1.0,
                                accum_out=stripS[:, kk - 1:kk],
                            )
                    continue
                ht = work.tile([128, FD], f32, tag="h")
                h = ht[:, :]
                nc.vector._custom_dve(
                    ops["poly3"], out=h, in0=psum[:, :],
                    s0=float(pA), s1=float(pB), imm2=float(pC),
                )
                nc.vector._custom_dve(
                    ops["recipacc"], out=dumpV[:, :], in0=h,
                    s0=float(w * RC0), s1=float(w * RC1),
                    imm2=float(w * RC2),
                    accum_out=stripV[:, k:k + 1],
                )
            nc.sync.dma_start(out[:, KPC:], stripS[:, :])
            nc.sync.dma_start(out[:, :KPC], stripV[:, :])
    nc.compile()
    return nc


def _feature_rows(pos, n2):
    """bf16 hi/lo-split feature rows: A (rhs, i-side) rows 0..12 and
    B (lhsT, j-side) rows 0..12; PSUM[j,i] = d2_ij up to dropped lo*lo."""
    import ml_dtypes
    bf16 = ml_dtypes.bfloat16
    ones = np.ones(len(pos), np.float32)

    def split(v):
        hi = v.astype(bf16).astype(np.float32)
        lo = (v - hi).astype(np.float32)
        return hi, lo

    phi, plo = split(pos)
    n2hi, n2lo = split(n2)
    A = np.stack([
        phi[:, 0], phi[:, 1], phi[:, 2],      # phi_i  . -2phi_j
        plo[:, 0], plo[:, 1], plo[:, 2],      # plo_i  . -2phi_j
        n2hi, n2lo,                           # n2_i   . 1
        ones, ones,                           # 1      . n2_j
        phi[:, 0], phi[:, 1], phi[:, 2],      # phi_i  . -2plo_j
    ])
    B = np.stack([
        -2 * phi[:, 0], -2 * phi[:, 1], -2 * phi[:, 2],
        -2 * phi[:, 0], -2 * phi[:, 1], -2 * phi[:, 2],
        ones, ones,
        n2hi, n2lo,
        -2 * plo[:, 0], -2 * plo[:, 1], -2 * plo[:, 2],
    ])
    return A.astype(bf16), B.astype(bf16)


def kernel(atomic_numbers=None, positions=None, r2r4=None, a1=None, a2=None,
           s6=None, s8=None):
    from concourse.bass_utils import run_bass_kernel_spmd

    pos = np.asarray(positions, np.float32)
    order = np.argsort(pos[:, 2], kind="stable")
    pos_s = pos[order]
    n2_s = (pos_s.astype(np.float64) ** 2).sum(-1).astype(np.float32)

    pA = CUB_A2 / CUB_A3
    pB = CUB_A1 / CUB_A3
    pC = CUB_A0 / CUB_A3

    import ml_dtypes
    bf16 = ml_dtypes.bfloat16
    Afeat, Bfeat = _feature_rows(pos_s, n2_s)

    # dummy block: j-side shifted far away -> d2 ~ 3e5, contribution ~ 0
    dpos = pos_s[:BLK] + np.float32(300.0)
    dn2 = (dpos.astype(np.float64) ** 2).sum(-1).astype(np.float32)
    _, Bdummy = _feature_rows(dpos, dn2)

    half = 3 * BLK
    per_core = _block_lists()
    in_maps = []
    for c in range(NCORES):
        ab = np.empty((NFEAT, 4 * half), dtype=bf16)
        for k, pair in enumerate(per_core[c]):
            base = (k // 3) * 2 * half
            sa = slice(base + (k % 3) * BLK, base + (k % 3 + 1) * BLK)
            sb = slice(base + half + (k % 3) * BLK,
                       base + half + (k % 3 + 1) * BLK)
            if pair is None:
                ab[:, sa] = Afeat[:, 0:BLK]
                ab[:, sb] = Bdummy
            else:
                bi, bj = pair
                ab[:, sa] = Afeat[:, bj * BLK:(bj + 1) * BLK]
                ab[:, sb] = Bfeat[:, bi * BLK:(bi + 1) * BLK]
        in_maps.append({"absel": np.ascontiguousarray(ab)})

    nc = _build_program(pA, pB, pC)

    import os
    import tempfile
    trace = bool(os.environ.get("BASS_PROFILE"))
    kw = {}
    if trace:
        kw = dict(trace=True, tmpdir=tempfile.mkdtemp(prefix="bass_prof_"))
    res = run_bass_kernel_spmd(nc, in_maps, list(range(NCORES)), **kw)
    global LAST_EXEC_NS, LAST_PROFILE, LAST_NC
    LAST_EXEC_NS = getattr(res, "exec_time_ns", None)
    LAST_PROFILE = getattr(res, "profile_json", None)
    LAST_NC = nc

    S = np.float64(0.0)
    for c in range(NCORES):
        S += np.asarray(res.results[c]["out"], np.float64).sum()
    # unmasked diagonal: each i==i pair contributes recip(H(~0)) with the
    # diag (w=1, VE-routed) instruction constants
    r0 = np.float64(_recip_cheb_host(np.float32(pC), 1.0 / CUB_A3))
    S -= np.float64(N) * r0
    return np.float32(-S)


if __name__ == "__main__":
    import reference
    inputs = reference.setup_inputs()
    outp = kernel(**{k: np.asarray(v) for k, v in inputs.items()})
    print("kernel:", outp)
